# revision 1
# baseline (speedup 1.0000x reference)
"""MoE router kernel for Trainium2 (8 NeuronCores, SPMD data-parallel).

Computes, for x [B,S,H] and gate_w [E,H]:
    logits = x @ gate_w.T           # [B,S,E]
    p = softmax(logits, -1)
    w, i = top_k(p, 2); w = w / w.sum(-1, keepdims=True)

Math used on-device: renormalized top-2 softmax weights collapse to
    w1 = sigmoid(l1 - l2), w2 = sigmoid(l2 - l1)
where l1 >= l2 are the top-2 logits, so the full softmax is never needed.

Sharding: tokens (B*S = 16384) split evenly across 8 cores; gate weights
replicated. Per core: 2048 tokens x 4096 hidden, fp32 throughout.

Per-core pipeline (128-token tiles):
  DMA x tile [128, 4096] -> PE transpose 128x128 blocks (matmul-by-identity,
  fp32) -> PSUM -> copy to SBUF (DVE/ACT alternating) -> fp32 GEMM vs
  pre-arranged gate_w.T chunks accumulating logits [128 tok, 64 e] in PSUM
  -> DVE max/max_index (top-8 sorted) -> ACT sigmoid -> DMA out.
"""

import sys

sys.path.insert(0, "/opt/trn_rl_repo")

import numpy as np

import concourse.bass as bass
import concourse.mybir as mybir
import concourse.tile as tile
from concourse.bass_utils import run_bass_kernel_spmd
import orjson
import concourse.bass_utils as _bu
import concourse.bass2jax as _b2j

_orig_compile_bir = _bu.compile_bir_kernel


def _legalize_waits(bir_json: bytes) -> bytes:
    """This walrus build allows only ONE sync-wait per compute
    instruction; move excess waits onto a Drain inserted just before
    (Drain accepts many waits)."""
    m = orjson.loads(bir_json)
    changed = False
    for fn in m["functions"]:
        for blk in fn["blocks"]:
            out = []
            for inst in blk["instructions"]:
                si = inst.get("sync_info")
                w = (si or {}).get("on_wait") or []
                if len(w) > 1:
                    for k, wk in enumerate(w[:-1]):
                        out.append({
                            "debug": inst.get("debug", 0),
                            "engine": inst["engine"],
                            "ins": [], "outs": [],
                            "name": inst["name"] + f"-lw{k}",
                            "opcode": "Drain",
                            "sync_info": {"on_update": [], "on_wait": [wk]},
                        })
                    si["on_wait"] = w[-1:]
                    changed = True
                out.append(inst)
            blk["instructions"] = out
    return orjson.dumps(m) if changed else bir_json


def _compile_bir_legalized(bir_json, tmpdir, neff_name="file.neff"):
    return _orig_compile_bir(_legalize_waits(bir_json), tmpdir, neff_name)


_bu.compile_bir_kernel = _compile_bir_legalized
_b2j.compile_bir_kernel = _compile_bir_legalized

F32 = mybir.dt.float32
U32 = mybir.dt.uint32

B, S, H, E = 4, 4096, 4096, 64
N_CORES = 8
P = 128                      # partitions / tile height
TOK_TOTAL = B * S            # 16384
TOK = TOK_TOTAL // N_CORES   # 2048 tokens per core
NCH = H // P                 # 32 contraction chunks of 128
GRP = 4                      # transpose chunks per PSUM bank (4*128*4B = 2KB)
NGRP = NCH // GRP            # 8 groups


def build_nc(tok: int = TOK):
    """Build the per-core Bass program (SPMD: same program, 8 cores)."""
    ntiles = tok // P
    nc = bass.Bass()

    x_ext = nc.declare_dram_parameter("x", [tok, H], F32, isOutput=False)
    wt_ext = nc.declare_dram_parameter("wt", [P, NCH, E], F32, isOutput=False)
    id_ext = nc.declare_dram_parameter("ident", [P, P], F32, isOutput=False)
    ow_ext = nc.declare_dram_parameter("out_w", [tok, 2], F32, isOutput=True)
    oi_ext = nc.declare_dram_parameter("out_i", [tok, 2], U32, isOutput=True)

    with tile.TileContext(nc) as tc:
        with (
            tc.tile_pool(name="consts", bufs=1) as consts,
            tc.tile_pool(name="xin", bufs=4) as xpool,
            tc.tile_pool(name="xt", bufs=4) as xtpool,
            tc.tile_pool(name="ps_t", bufs=4, space="PSUM") as ps_t,
            tc.tile_pool(name="ps_l", bufs=2, space="PSUM") as ps_l,
            tc.tile_pool(name="small", bufs=3) as small,
            tc.tile_pool(name="outp", bufs=3) as outp,
        ):
            wt_sb = consts.tile([P, NCH, E], F32)
            nc.sync.dma_start(wt_sb[:], wt_ext[:])
            id_sb = consts.tile([P, P], F32)
            nc.sync.dma_start(id_sb[:], id_ext[:])

            # Primers: walrus allows only ONE sync-wait per compute
            # instruction. Each engine's first instruction carries a
            # preamble self-guard wait, and the fused fp32 LDWEIGHTS can
            # hold just one wait total - so give every engine a first op
            # with no other dependency (const APs are pre-TileContext,
            # untracked), and absorb each const-DMA sem into its own
            # throwaway PE op.
            prim = consts.tile([P, 2], F32)
            nc.vector.memset(prim[:, 0:1], 0.0)
            nc.scalar.copy(prim[:, 1:2], nc.const_aps.tensor(1.0, (P, 1)))
            with tc.tile_pool(name="scr", bufs=1, space="PSUM") as scr_pool:
                scr = scr_pool.tile([P, P], F32)
                nc.tensor.matmul(scr[:], id_sb[:], id_sb[:],
                                 is_transpose=True, start=True, stop=True)
                nc.tensor.matmul(scr[0:E, :], wt_sb[:, 0, :], id_sb[:],
                                 is_transpose=True, start=True, stop=True)

            for t in range(ntiles):
                x_sb = xpool.tile([P, H], F32)
                nc.sync.dma_start(x_sb[:], x_ext[t * P:(t + 1) * P, :])

                logits_ps = ps_l.tile([P, E], F32)
                for g in range(NGRP):
                    # Transpose 4 chunks of x into one PSUM bank:
                    # x[tok, 128c:128c+128] -> xT[h 128, tok 128]
                    xT_ps = ps_t.tile([P, GRP, P], F32)
                    for j in range(GRP):
                        c = GRP * g + j
                        nc.tensor.matmul(
                            xT_ps[:, j, :],
                            x_sb[:, c * P:(c + 1) * P],
                            id_sb[:],
                            is_transpose=True,
                            start=(j == 0),
                            stop=(j == GRP - 1),
                        )
                    xT_sb = xtpool.tile([P, GRP, P], F32)
                    if g % 2 == 0:
                        nc.vector.tensor_copy(xT_sb[:], xT_ps[:])
                    else:
                        nc.scalar.copy(xT_sb[:], xT_ps[:])
                    # GEMM: logits[tok, e] += xT.T @ wT   (contraction over h)
                    for j in range(GRP):
                        c = GRP * g + j
                        nc.tensor.matmul(
                            logits_ps[:],
                            xT_sb[:, j, :],
                            wt_sb[:, c, :],
                            start=(c == 0),
                            stop=(c == NCH - 1),
                        )

                lg = small.tile([P, E], F32)
                nc.vector.tensor_copy(lg[:], logits_ps[:])
                mx = small.tile([P, 8], F32)
                nc.vector.max(mx[:], lg[:])
                ix = small.tile([P, 8], U32)
                nc.vector.max_index(ix[:], mx[:], lg[:])

                ow_t = outp.tile([P, 2], F32)
                oi_t = outp.tile([P, 2], U32)
                # w1 = sigmoid(l1 - l2) = sigmoid(-1*l2 + l1); w2 symmetric
                nc.scalar.activation(
                    ow_t[:, 0:1], mx[:, 1:2],
                    mybir.ActivationFunctionType.Sigmoid,
                    bias=mx[:, 0:1], scale=-1.0,
                )
                nc.scalar.activation(
                    ow_t[:, 1:2], mx[:, 0:1],
                    mybir.ActivationFunctionType.Sigmoid,
                    bias=mx[:, 1:2], scale=-1.0,
                )
                nc.vector.tensor_copy(oi_t[:], ix[:, 0:2])

                nc.sync.dma_start(ow_ext[t * P:(t + 1) * P, :], ow_t[:])
                nc.sync.dma_start(oi_ext[t * P:(t + 1) * P, :], oi_t[:])

    return nc


_NC_CACHE = {}


def _get_nc(tok: int):
    if tok not in _NC_CACHE:
        _NC_CACHE[tok] = build_nc(tok)
    return _NC_CACHE[tok]


def make_in_maps(x: np.ndarray, gate_w: np.ndarray):
    """Shard full inputs into per-core input maps."""
    xf = np.ascontiguousarray(x.reshape(TOK_TOTAL, H), dtype=np.float32)
    # wt[p, c, e] = gate_w[e, 128*c + p]
    wt = np.ascontiguousarray(
        gate_w.T.reshape(NCH, P, E).transpose(1, 0, 2), dtype=np.float32
    )
    ident = np.eye(P, dtype=np.float32)
    return [
        {"x": np.ascontiguousarray(xf[i * TOK:(i + 1) * TOK]),
         "wt": wt, "ident": ident}
        for i in range(N_CORES)
    ]


def kernel(x, gate_w, _trace: bool = False):
    x = np.asarray(x, dtype=np.float32)
    gate_w = np.asarray(gate_w, dtype=np.float32)
    nc = _get_nc(TOK)
    in_maps = make_in_maps(x, gate_w)
    res = run_bass_kernel_spmd(
        nc, in_maps, core_ids=list(range(N_CORES)), trace=_trace
    )
    out_w = np.concatenate([res.results[i]["out_w"] for i in range(N_CORES)])
    out_i = np.concatenate([res.results[i]["out_i"] for i in range(N_CORES)])
    topk_weights = out_w.reshape(B, S, 2)
    topk_indices = out_i.astype(np.int32).reshape(B, S, 2)
    if _trace:
        kernel._last_result = res
    return topk_weights, topk_indices



# revision 4
# speedup vs baseline: 1.3364x; 1.3364x over previous
"""MoE router kernel for Trainium2 (8 NeuronCores, SPMD data-parallel).

Computes, for x [B,S,H] and gate_w [E,H]:
    logits = x @ gate_w.T           # [B,S,E]
    p = softmax(logits, -1)
    w, i = top_k(p, 2); w = w / w.sum(-1, keepdims=True)

Math used on-device: renormalized top-2 softmax weights collapse to
    w1 = sigmoid(l1 - l2), w2 = 1 - w1
where l1 >= l2 are the top-2 logits, so the full softmax is never needed.

Sharding: tokens (B*S = 16384) split evenly across 8 cores; gate weights
replicated. Per core: 2048 tokens x 4096 hidden.

v2 design (weight-stationary, compensated fp32r):
  Host pre-transposes each core's x slice to [H, tok] (pure layout), so
  no on-device transposes of x are needed. Per 128-row contraction chunk:
  DMA xT chunk [128, 2048] -> ACT rounds to fp32r (x_r) -> DVE computes
  the fp32r residual e_x = x - x_r -> PE accumulates logitsT [64, tok]
  in PSUM with three fp32r matmuls per chunk (w_r*x_r + w_r*e_x + w_e*x_r,
  1 cycle/row at moving dim 512), giving fp32-grade logits at ~4x the
  fp32 matmul rate. Tail: drain logitsT, PE-transpose 128-token tiles
  back to [tok, 64], DVE max8/max_index, one batched sigmoid.
"""

import sys

sys.path.insert(0, "/opt/trn_rl_repo")

import numpy as np

import concourse.bass as bass
import concourse.mybir as mybir
import concourse.tile as tile
from concourse.bass_utils import run_bass_kernel_spmd
import orjson
import concourse.bass_utils as _bu
import concourse.bass2jax as _b2j

_orig_compile_bir = _bu.compile_bir_kernel


def _legalize_waits(bir_json: bytes) -> bytes:
    """This walrus build allows only ONE sync-wait per compute
    instruction; move excess waits onto a Drain inserted just before
    (Drain accepts many waits)."""
    m = orjson.loads(bir_json)
    changed = False
    for fn in m["functions"]:
        for blk in fn["blocks"]:
            out = []
            for inst in blk["instructions"]:
                si = inst.get("sync_info")
                w = (si or {}).get("on_wait") or []
                if len(w) > 1:
                    for k, wk in enumerate(w[:-1]):
                        out.append({
                            "debug": inst.get("debug", 0),
                            "engine": inst["engine"],
                            "ins": [], "outs": [],
                            "name": inst["name"] + f"-lw{k}",
                            "opcode": "Drain",
                            "sync_info": {"on_update": [], "on_wait": [wk]},
                        })
                    si["on_wait"] = w[-1:]
                    changed = True
                out.append(inst)
            blk["instructions"] = out
    return orjson.dumps(m) if changed else bir_json


def _compile_bir_legalized(bir_json, tmpdir, neff_name="file.neff"):
    return _orig_compile_bir(_legalize_waits(bir_json), tmpdir, neff_name)


_bu.compile_bir_kernel = _compile_bir_legalized
_b2j.compile_bir_kernel = _compile_bir_legalized

F32 = mybir.dt.float32
F32R = mybir.dt.float32r
U32 = mybir.dt.uint32
Alu = mybir.AluOpType

B, S, H, E = 4, 4096, 4096, 64
N_CORES = 8
P = 128                      # partitions / tile height
TOK_TOTAL = B * S            # 16384
TOK = TOK_TOTAL // N_CORES   # 2048 tokens per core
NCH = H // P                 # 32 contraction chunks of 128
TB = 512                     # tokens per PSUM bank (fp32r moving max)
NB = TOK // TB               # 4 logitsT banks
NT = TOK // P                # 16 output tiles of 128 tokens


def build_nc(tok: int = TOK):
    """Build the per-core Bass program (SPMD: same program, 8 cores)."""
    nb = tok // TB
    nt = tok // P
    nc = bass.Bass()

    xt_ext = nc.declare_dram_parameter("xt", [NCH, P, tok], F32,
                                       isOutput=False)
    wt_ext = nc.declare_dram_parameter("wt", [P, NCH, E], F32, isOutput=False)
    id_ext = nc.declare_dram_parameter("ident", [P, P], F32, isOutput=False)
    ow_ext = nc.declare_dram_parameter("out_w", [tok, 2], F32, isOutput=True)
    oi_ext = nc.declare_dram_parameter("out_i", [tok, 2], U32, isOutput=True)

    with tile.TileContext(nc) as tc:
        with (
            tc.tile_pool(name="consts", bufs=1) as consts,
            tc.tile_pool(name="xin", bufs=4) as xin,
            tc.tile_pool(name="xr", bufs=4) as xrp,
            tc.tile_pool(name="ex", bufs=4) as exp_,
            tc.tile_pool(name="psl", bufs=1, space="PSUM") as psl,
            tc.tile_pool(name="pst", bufs=3, space="PSUM") as pst,
            tc.tile_pool(name="small", bufs=4) as small,
            tc.tile_pool(name="outp", bufs=1) as outp,
        ):
            wt_sb = consts.tile([P, NCH, E], F32)
            nc.sync.dma_start(wt_sb[:], wt_ext[:])
            id_sb = consts.tile([P, P], F32)
            nc.sync.dma_start(id_sb[:], id_ext[:])

            # Primers: walrus allows only ONE sync-wait per compute
            # instruction. Each engine's first instruction carries a
            # preamble self-guard wait, and the fused fp32r LDWEIGHTS can
            # hold just one wait total - so give every engine a first op
            # with no other dependency (const APs are pre-TileContext,
            # untracked), and absorb each const-DMA sem into its own
            # throwaway PE op.
            prim = consts.tile([P, 2], F32)
            nc.vector.memset(prim[:, 0:1], 0.0)
            nc.scalar.copy(prim[:, 1:2], nc.const_aps.tensor(1.0, (P, 1)))
            with tc.tile_pool(name="scr", bufs=1, space="PSUM") as scr_pool:
                scr = scr_pool.tile([P, P], F32)
                nc.tensor.matmul(scr[:], id_sb[:], id_sb[:],
                                 is_transpose=True, start=True, stop=True)
                nc.tensor.matmul(scr[0:E, :], wt_sb[:, 0, :], id_sb[:],
                                 is_transpose=True, start=True, stop=True)

            # Round gate weights to fp32r + residual (one-time, tiny).
            w_r = consts.tile([P, NCH, E], F32R)
            nc.vector.tensor_copy(w_r[:], wt_sb[:])
            w_e = consts.tile([P, NCH, E], F32R)
            nc.vector.scalar_tensor_tensor(
                w_e[:], wt_sb[:], 1.0, w_r[:].bitcast(F32),
                Alu.mult, Alu.subtract)

            # logitsT accumulators: nb banks of [64, TB].
            lgT = [psl.tile([E, TB], F32, name=f"lgT{b}") for b in range(nb)]

            for c in range(NCH):
                x_sb = xin.tile([P, tok], F32)
                nc.sync.dma_start(x_sb[:], xt_ext[c])
                x_r = xrp.tile([P, tok], F32R)
                nc.scalar.copy(x_r[:], x_sb[:])        # ACT: round to fp32r
                e_x = exp_.tile([P, tok], F32R)
                nc.vector.scalar_tensor_tensor(        # DVE: residual
                    e_x[:], x_sb[:], 1.0, x_r[:].bitcast(F32),
                    Alu.mult, Alu.subtract)
                for b in range(nb):
                    nc.tensor.matmul(lgT[b][:], w_r[:, c, :],
                                     x_r[:, b * TB:(b + 1) * TB],
                                     start=(c == 0), stop=False)
                for b in range(nb):
                    nc.tensor.matmul(lgT[b][:], w_r[:, c, :],
                                     e_x[:, b * TB:(b + 1) * TB],
                                     start=False, stop=False)
                for b in range(nb):
                    nc.tensor.matmul(lgT[b][:], w_e[:, c, :],
                                     x_r[:, b * TB:(b + 1) * TB],
                                     start=False, stop=(c == NCH - 1))

            # Tail: drain logitsT -> SBUF, transpose back per 128-token
            # tile, top-2 + batched sigmoid.
            mxa = outp.tile([P, nt, 2], F32)
            ixa = outp.tile([P, nt, 2], U32)
            for b in range(nb):
                lgT_sb = small.tile([E, TB], F32)
                if b % 2 == 0:
                    nc.scalar.copy(lgT_sb[:], lgT[b][:])
                else:
                    nc.vector.tensor_copy(lgT_sb[:], lgT[b][:])
                for j in range(TB // P):
                    t = b * (TB // P) + j
                    lg_ps = pst.tile([P, E], F32)
                    nc.tensor.matmul(lg_ps[:], lgT_sb[:, j * P:(j + 1) * P],
                                     id_sb[0:E, 0:E], is_transpose=True,
                                     start=True, stop=True)
                    mx = small.tile([P, 8], F32)
                    nc.vector.max(mx[:], lg_ps[:])
                    ix = small.tile([P, 8], U32)
                    nc.vector.max_index(ix[:], mx[:], lg_ps[:])
                    nc.vector.tensor_copy(mxa[:, t, :], mx[:, 0:2])
                    nc.vector.tensor_copy(ixa[:, t, :], ix[:, 0:2])

            # w1 = sigmoid(l1 - l2), w2 = 1 - w1, batched over all tiles.
            d = outp.tile([P, nt], F32)
            nc.vector.scalar_tensor_tensor(
                d[:], mxa[:, :, 0], 1.0, mxa[:, :, 1], Alu.mult, Alu.subtract)
            owa = outp.tile([P, nt, 2], F32)
            nc.scalar.activation(owa[:, :, 0], d[:],
                                 mybir.ActivationFunctionType.Sigmoid)
            nc.vector.tensor_scalar(owa[:, :, 1], owa[:, :, 0], -1.0, 1.0,
                                    Alu.mult, Alu.add)

            for t in range(nt):
                eng = nc.sync if t % 2 == 0 else nc.scalar
                eng.dma_start(ow_ext[t * P:(t + 1) * P, :], owa[:, t, :])
                eng.dma_start(oi_ext[t * P:(t + 1) * P, :], ixa[:, t, :])

    return nc


_NC_CACHE = {}


def _get_nc(tok: int):
    if tok not in _NC_CACHE:
        _NC_CACHE[tok] = build_nc(tok)
    return _NC_CACHE[tok]


def make_in_maps(x: np.ndarray, gate_w: np.ndarray):
    """Shard full inputs into per-core input maps (layout only)."""
    xf = np.ascontiguousarray(x.reshape(TOK_TOTAL, H), dtype=np.float32)
    # wt[p, c, e] = gate_w[e, 128*c + p]
    wt = np.ascontiguousarray(
        gate_w.T.reshape(NCH, P, E).transpose(1, 0, 2), dtype=np.float32
    )
    ident = np.eye(P, dtype=np.float32)
    maps = []
    for i in range(N_CORES):
        xc = xf[i * TOK:(i + 1) * TOK]              # [tok, H]
        xt = np.ascontiguousarray(xc.T).reshape(NCH, P, TOK)
        maps.append({"xt": xt, "wt": wt, "ident": ident})
    return maps


def kernel(x, gate_w, _trace: bool = False):
    x = np.asarray(x, dtype=np.float32)
    gate_w = np.asarray(gate_w, dtype=np.float32)
    nc = _get_nc(TOK)
    in_maps = make_in_maps(x, gate_w)
    res = run_bass_kernel_spmd(
        nc, in_maps, core_ids=list(range(N_CORES)), trace=_trace
    )
    out_w = np.concatenate([res.results[i]["out_w"] for i in range(N_CORES)])
    out_i = np.concatenate([res.results[i]["out_i"] for i in range(N_CORES)])
    topk_weights = out_w.reshape(B, S, 2)
    topk_indices = out_i.astype(np.int32).reshape(B, S, 2)
    if _trace:
        kernel._last_result = res
    return topk_weights, topk_indices


# revision 6
# speedup vs baseline: 1.6016x; 1.1984x over previous
"""MoE router kernel for Trainium2 (8 NeuronCores, SPMD data-parallel).

Computes, for x [B,S,H] and gate_w [E,H]:
    logits = x @ gate_w.T           # [B,S,E]
    p = softmax(logits, -1)
    w, i = top_k(p, 2); w = w / w.sum(-1, keepdims=True)

Math used on-device: renormalized top-2 softmax weights collapse to
    w1 = sigmoid(l1 - l2), w2 = 1 - w1
where l1 >= l2 are the top-2 logits, so the full softmax is never needed.

Sharding: tokens (B*S = 16384) split evenly across 8 cores; gate weights
replicated. Per core: 2048 tokens x 4096 hidden.

v3 design (weight-stationary, split-precision bf16):
  The host splits x and gate_w into bf16 hi/lo pairs (x = xh + xl to 16
  mantissa bits) and pre-transposes each core's x slice to [H, tok]
  layout, packed as [chunk, 128, {hi,lo}, tok]. On device, per 128-row
  contraction chunk: one DMA brings both halves; PE accumulates logitsT
  [64, tok] in PSUM with three bf16 matmuls per chunk
  (wh*xh + wh*xl + wl*xh, 1 cycle/row, moving dim 512), giving
  fp32-grade logits (sigma ~ 4e-6, verified zero top-2 flips on the
  problem distribution). No vector/scalar-engine work in the stream.
  Tail: drain logitsT, PE-transpose back to [tok, 64] tiles, DVE
  max8/max_index, one batched sigmoid, single packed output DMA per
  tensor (host unpermutes).
"""

import sys

sys.path.insert(0, "/opt/trn_rl_repo")

import numpy as np
import ml_dtypes

import concourse.bass as bass
import concourse.mybir as mybir
import concourse.tile as tile
from concourse.bass_utils import run_bass_kernel_spmd
import orjson
import concourse.bass_utils as _bu
import concourse.bass2jax as _b2j

_orig_compile_bir = _bu.compile_bir_kernel


def _legalize_waits(bir_json: bytes) -> bytes:
    """This walrus build allows only ONE sync-wait per compute
    instruction; move excess waits onto a Drain inserted just before
    (Drain accepts many waits)."""
    m = orjson.loads(bir_json)
    changed = False
    for fn in m["functions"]:
        for blk in fn["blocks"]:
            out = []
            for inst in blk["instructions"]:
                si = inst.get("sync_info")
                w = (si or {}).get("on_wait") or []
                if len(w) > 1:
                    for k, wk in enumerate(w[:-1]):
                        out.append({
                            "debug": inst.get("debug", 0),
                            "engine": inst["engine"],
                            "ins": [], "outs": [],
                            "name": inst["name"] + f"-lw{k}",
                            "opcode": "Drain",
                            "sync_info": {"on_update": [], "on_wait": [wk]},
                        })
                    si["on_wait"] = w[-1:]
                    changed = True
                out.append(inst)
            blk["instructions"] = out
    return orjson.dumps(m) if changed else bir_json


def _compile_bir_legalized(bir_json, tmpdir, neff_name="file.neff"):
    return _orig_compile_bir(_legalize_waits(bir_json), tmpdir, neff_name)


_bu.compile_bir_kernel = _compile_bir_legalized
_b2j.compile_bir_kernel = _compile_bir_legalized

F32 = mybir.dt.float32
BF16 = mybir.dt.bfloat16
U32 = mybir.dt.uint32
Alu = mybir.AluOpType
BF = ml_dtypes.bfloat16

B, S, H, E = 4, 4096, 4096, 64
N_CORES = 8
P = 128                      # partitions / tile height
TOK_TOTAL = B * S            # 16384
TOK = TOK_TOTAL // N_CORES   # 2048 tokens per core
NCH = H // P                 # 32 contraction chunks of 128
TB = 512                     # tokens per PSUM bank
NB = TOK // TB               # 4 logitsT banks
NT = TOK // P                # 16 output tiles of 128 tokens


def build_nc(tok: int = TOK):
    """Build the per-core Bass program (SPMD: same program, 8 cores)."""
    nb = tok // TB
    nt = tok // P
    nc = bass.Bass()

    xt_ext = nc.declare_dram_parameter("xt", [NCH, P, 2, tok], BF16,
                                       isOutput=False)
    wt_ext = nc.declare_dram_parameter("wt", [P, NCH, 2, E], BF16,
                                       isOutput=False)
    id_ext = nc.declare_dram_parameter("ident", [P, P], F32, isOutput=False)
    ow_ext = nc.declare_dram_parameter("out_w", [P, nt, 2], F32,
                                       isOutput=True)
    oi_ext = nc.declare_dram_parameter("out_i", [P, nt, 2], U32,
                                       isOutput=True)

    with tile.TileContext(nc) as tc:
        with (
            tc.tile_pool(name="consts", bufs=1) as consts,
            tc.tile_pool(name="xin", bufs=4) as xin,
            tc.tile_pool(name="psl", bufs=1, space="PSUM") as psl,
            tc.tile_pool(name="pst", bufs=2, space="PSUM") as pst,
            tc.tile_pool(name="small", bufs=4) as small,
            tc.tile_pool(name="outp", bufs=1) as outp,
        ):
            wt_sb = consts.tile([P, NCH, 2, E], BF16)
            nc.sync.dma_start(wt_sb[:], wt_ext[:])
            id_sb = consts.tile([P, P], F32)
            nc.sync.dma_start(id_sb[:], id_ext[:])

            # Primers: walrus allows only ONE sync-wait per compute
            # instruction. Give every engine a first op with no other
            # dependency (const APs are pre-TileContext, untracked), and
            # absorb each const-DMA sem into its own throwaway PE op.
            prim = consts.tile([P, 2], F32)
            nc.vector.memset(prim[:, 0:1], 0.0)
            nc.scalar.copy(prim[:, 1:2], nc.const_aps.tensor(1.0, (P, 1)))
            with tc.tile_pool(name="scr", bufs=1, space="PSUM") as scr_pool:
                scr = scr_pool.tile([P, P], F32)
                nc.tensor.matmul(scr[:], id_sb[:], id_sb[:],
                                 is_transpose=True, start=True, stop=True)
                scr2 = scr_pool.tile([E, P], BF16)
                nc.tensor.matmul(scr2[:], wt_sb[:, 0, 0, :],
                                 id_sb[:].bitcast(BF16)[:, 0:P],
                                 is_transpose=True, start=True, stop=True)

            # logitsT accumulators: nb banks of [64, TB].
            lgT = [psl.tile([E, TB], F32, name=f"lgT{b}") for b in range(nb)]

            for c in range(NCH):
                x_sb = xin.tile([P, 2, tok], BF16)
                nc.sync.dma_start(x_sb[:], xt_ext[c])
                for b in range(nb):
                    nc.tensor.matmul(lgT[b][:], wt_sb[:, c, 0, :],
                                     x_sb[:, 0, b * TB:(b + 1) * TB],
                                     start=(c == 0), stop=False)
                for b in range(nb):
                    nc.tensor.matmul(lgT[b][:], wt_sb[:, c, 0, :],
                                     x_sb[:, 1, b * TB:(b + 1) * TB],
                                     start=False, stop=False)
                for b in range(nb):
                    nc.tensor.matmul(lgT[b][:], wt_sb[:, c, 1, :],
                                     x_sb[:, 0, b * TB:(b + 1) * TB],
                                     start=False, stop=(c == NCH - 1))

            # Tail: drain logitsT -> SBUF, transpose back per 128-token
            # tile, top-2 + batched sigmoid.
            mxa = outp.tile([P, nt, 2], F32)
            ixa = outp.tile([P, nt, 2], U32)
            for b in range(nb):
                lgT_sb = small.tile([E, TB], F32)
                if b % 2 == 0:
                    nc.scalar.copy(lgT_sb[:], lgT[b][:])
                else:
                    nc.vector.tensor_copy(lgT_sb[:], lgT[b][:])
                for j in range(TB // P):
                    t = b * (TB // P) + j
                    lg_ps = pst.tile([P, E], F32)
                    nc.tensor.matmul(lg_ps[:], lgT_sb[:, j * P:(j + 1) * P],
                                     id_sb[0:E, 0:E], is_transpose=True,
                                     start=True, stop=True)
                    mx = small.tile([P, 8], F32)
                    nc.vector.max(mx[:], lg_ps[:])
                    ix = small.tile([P, 8], U32)
                    nc.vector.max_index(ix[:], mx[:], lg_ps[:])
                    nc.vector.tensor_copy(mxa[:, t, :], mx[:, 0:2])
                    nc.vector.tensor_copy(ixa[:, t, :], ix[:, 0:2])

            # w1 = sigmoid(l1 - l2), w2 = 1 - w1, batched over all tiles.
            d = outp.tile([P, nt], F32)
            nc.vector.scalar_tensor_tensor(
                d[:], mxa[:, :, 0], 1.0, mxa[:, :, 1], Alu.mult, Alu.subtract)
            owa = outp.tile([P, nt, 2], F32)
            nc.scalar.activation(owa[:, :, 0], d[:],
                                 mybir.ActivationFunctionType.Sigmoid)
            nc.vector.tensor_scalar(owa[:, :, 1], owa[:, :, 0], -1.0, 1.0,
                                    Alu.mult, Alu.add)

            nc.sync.dma_start(ow_ext[:], owa[:])
            nc.scalar.dma_start(oi_ext[:], ixa[:])

    return nc


_NC_CACHE = {}


def _get_nc(tok: int):
    if tok not in _NC_CACHE:
        _NC_CACHE[tok] = build_nc(tok)
    return _NC_CACHE[tok]


def make_in_maps(x: np.ndarray, gate_w: np.ndarray):
    """Shard + split inputs into per-core input maps."""
    xf = np.ascontiguousarray(x.reshape(TOK_TOTAL, H), dtype=np.float32)
    xh = xf.astype(BF)
    xl = (xf - xh.astype(np.float32)).astype(BF)

    w32 = np.asarray(gate_w, np.float32)
    wh = w32.astype(BF)
    wl = (w32 - wh.astype(np.float32)).astype(BF)
    # wt[p, c, s, e] = w_s[e, 128*c + p]
    wt = np.empty((P, NCH, 2, E), dtype=BF)
    wt[:, :, 0, :] = wh.T.reshape(NCH, P, E).transpose(1, 0, 2)
    wt[:, :, 1, :] = wl.T.reshape(NCH, P, E).transpose(1, 0, 2)

    ident = np.eye(P, dtype=np.float32)
    maps = []
    for i in range(N_CORES):
        sl = slice(i * TOK, (i + 1) * TOK)
        # xt[c, p, s, t] = x_s[token t, 128c + p]
        xt = np.empty((NCH, P, 2, TOK), dtype=BF)
        xt[:, :, 0, :] = np.ascontiguousarray(xh[sl].T).reshape(NCH, P, TOK)
        xt[:, :, 1, :] = np.ascontiguousarray(xl[sl].T).reshape(NCH, P, TOK)
        maps.append({"xt": xt, "wt": wt, "ident": ident})
    return maps


def kernel(x, gate_w, _trace: bool = False):
    x = np.asarray(x, dtype=np.float32)
    gate_w = np.asarray(gate_w, dtype=np.float32)
    nc = _get_nc(TOK)
    in_maps = make_in_maps(x, gate_w)
    res = run_bass_kernel_spmd(
        nc, in_maps, core_ids=list(range(N_CORES)), trace=_trace
    )
    # Device returns [128, nt, 2] partition-major; unpermute to [tok, 2].
    out_w = np.concatenate([
        res.results[i]["out_w"].transpose(1, 0, 2).reshape(TOK, 2)
        for i in range(N_CORES)
    ])
    out_i = np.concatenate([
        res.results[i]["out_i"].transpose(1, 0, 2).reshape(TOK, 2)
        for i in range(N_CORES)
    ])
    topk_weights = out_w.reshape(B, S, 2)
    topk_indices = out_i.astype(np.int32).reshape(B, S, 2)
    if _trace:
        kernel._last_result = res
    return topk_weights, topk_indices


# revision 11
# speedup vs baseline: 1.6553x; 1.0336x over previous
"""MoE router kernel for Trainium2 (8 NeuronCores, SPMD data-parallel).

Computes, for x [B,S,H] and gate_w [E,H]:
    logits = x @ gate_w.T           # [B,S,E]
    p = softmax(logits, -1)
    w, i = top_k(p, 2); w = w / w.sum(-1, keepdims=True)

Math used on-device: renormalized top-2 softmax weights collapse to
    w1 = sigmoid(l1 - l2), w2 = 1 - w1
where l1 >= l2 are the top-2 logits, so the full softmax is never needed.

Sharding: tokens (B*S = 16384) split evenly across 8 cores; gate weights
replicated. Per core: 2048 tokens x 4096 hidden.

v3 design (weight-stationary, split-precision bf16):
  The host splits x and gate_w into bf16 hi/lo pairs (x = xh + xl to 16
  mantissa bits) and pre-transposes each core's x slice to [H, tok]
  layout, packed as [chunk, 128, {hi,lo}, tok]. On device, per 128-row
  contraction chunk: one DMA brings both halves; PE accumulates logitsT
  [64, tok] in PSUM with three bf16 matmuls per chunk
  (wh*xh + wh*xl + wl*xh, 1 cycle/row, moving dim 512), giving
  fp32-grade logits (sigma ~ 4e-6, verified zero top-2 flips on the
  problem distribution). No vector/scalar-engine work in the stream.
  Tail: drain logitsT, PE-transpose back to [tok, 64] tiles, DVE
  max8/max_index, one batched sigmoid, single packed output DMA per
  tensor (host unpermutes).
"""

import sys

sys.path.insert(0, "/opt/trn_rl_repo")

import numpy as np
import ml_dtypes

import concourse.bass as bass
import concourse.mybir as mybir
import concourse.tile as tile
from concourse.bass_utils import run_bass_kernel_spmd
import orjson
import concourse.bass_utils as _bu
import concourse.bass2jax as _b2j

_orig_compile_bir = _bu.compile_bir_kernel


def _legalize_waits(bir_json: bytes) -> bytes:
    """This walrus build allows only ONE sync-wait per compute
    instruction; move excess waits onto a Drain inserted just before
    (Drain accepts many waits)."""
    m = orjson.loads(bir_json)
    changed = False
    for fn in m["functions"]:
        for blk in fn["blocks"]:
            out = []
            for inst in blk["instructions"]:
                si = inst.get("sync_info")
                w = (si or {}).get("on_wait") or []
                if len(w) > 1:
                    for k, wk in enumerate(w[:-1]):
                        out.append({
                            "debug": inst.get("debug", 0),
                            "engine": inst["engine"],
                            "ins": [], "outs": [],
                            "name": inst["name"] + f"-lw{k}",
                            "opcode": "Drain",
                            "sync_info": {"on_update": [], "on_wait": [wk]},
                        })
                    si["on_wait"] = w[-1:]
                    changed = True
                out.append(inst)
            blk["instructions"] = out
    return orjson.dumps(m) if changed else bir_json


def _compile_bir_legalized(bir_json, tmpdir, neff_name="file.neff"):
    return _orig_compile_bir(_legalize_waits(bir_json), tmpdir, neff_name)


_bu.compile_bir_kernel = _compile_bir_legalized
_b2j.compile_bir_kernel = _compile_bir_legalized

F32 = mybir.dt.float32
BF16 = mybir.dt.bfloat16
U32 = mybir.dt.uint32
Alu = mybir.AluOpType
BF = ml_dtypes.bfloat16

B, S, H, E = 4, 4096, 4096, 64
N_CORES = 8
P = 128                      # partitions / tile height
TOK_TOTAL = B * S            # 16384
TOK = TOK_TOTAL // N_CORES   # 2048 tokens per core
NCH = H // P                 # 32 contraction chunks of 128
TB = 512                     # tokens per PSUM bank
NB = TOK // TB               # 4 logitsT banks
NT = TOK // P                # 16 output tiles of 128 tokens


def build_nc(tok: int = TOK):
    """Build the per-core Bass program (SPMD: same program, 8 cores)."""
    nb = tok // TB
    nt = tok // P
    nc = bass.Bass()

    nsc = NCH // 2
    xt_ext = nc.declare_dram_parameter("xt", [nsc, P, 2, 2, tok], BF16,
                                       isOutput=False)
    wt_ext = nc.declare_dram_parameter("wt", [P, NCH, 2, E], BF16,
                                       isOutput=False)
    id_ext = nc.declare_dram_parameter("ident", [P, P], F32, isOutput=False)
    ow_ext = nc.declare_dram_parameter("out_w", [P, nt, 2], F32,
                                       isOutput=True)
    oi_ext = nc.declare_dram_parameter("out_i", [P, nt, 8], U32,
                                       isOutput=True)

    with tile.TileContext(nc) as tc:
        with (
            tc.tile_pool(name="consts", bufs=1) as consts,
            tc.tile_pool(name="xin", bufs=4) as xin,
            tc.tile_pool(name="psl", bufs=1, space="PSUM") as psl,
            tc.tile_pool(name="pst", bufs=2, space="PSUM") as pst,
            tc.tile_pool(name="small", bufs=4) as small,
            tc.tile_pool(name="outp", bufs=1) as outp,
        ):
            wt_sb = consts.tile([P, NCH, 2, E], BF16)
            nc.scalar.dma_start(wt_sb[:], wt_ext[:])
            id_sb = consts.tile([P, P], F32)
            nc.scalar.dma_start(id_sb[:], id_ext[:])

            # Primers: walrus allows only ONE sync-wait per compute
            # instruction. Give every engine a first op with no other
            # dependency (const APs are pre-TileContext, untracked), and
            # absorb each const-DMA sem into its own throwaway PE op.
            prim = consts.tile([P, 2], F32)
            nc.vector.memset(prim[:, 0:1], 0.0)
            nc.scalar.copy(prim[:, 1:2], nc.const_aps.tensor(1.0, (P, 1)))
            with tc.tile_pool(name="scr", bufs=1, space="PSUM") as scr_pool:
                scr = scr_pool.tile([P, P], F32)
                nc.tensor.matmul(scr[:], id_sb[:], id_sb[:],
                                 is_transpose=True, start=True, stop=True)
                scr2 = scr_pool.tile([E, P], BF16)
                nc.tensor.matmul(scr2[:], wt_sb[:, 0, 0, :],
                                 id_sb[:].bitcast(BF16)[:, 0:P],
                                 is_transpose=True, start=True, stop=True)

            # logitsT accumulators: nb banks of [64, TB].
            lgT = [psl.tile([E, TB], F32, name=f"lgT{b}") for b in range(nb)]

            for sc in range(nsc):
                x_sb = xin.tile([P, 2, 2, tok], BF16)
                nc.sync.dma_start(x_sb[:], xt_ext[sc])
                for cc in range(2):
                    c = 2 * sc + cc
                    for b in range(nb):
                        nc.tensor.matmul(lgT[b][:], wt_sb[:, c, 0, :],
                                         x_sb[:, cc, 0, b * TB:(b + 1) * TB],
                                         start=(c == 0), stop=False)
                    for b in range(nb):
                        nc.tensor.matmul(lgT[b][:], wt_sb[:, c, 0, :],
                                         x_sb[:, cc, 1, b * TB:(b + 1) * TB],
                                         start=False, stop=False)
                    for b in range(nb):
                        nc.tensor.matmul(lgT[b][:], wt_sb[:, c, 1, :],
                                         x_sb[:, cc, 0, b * TB:(b + 1) * TB],
                                         start=False, stop=(c == NCH - 1))

            # Tail: drain logitsT -> SBUF, transpose back per 128-token
            # tile, top-2 + batched sigmoid. max8/max_index write straight
            # into batched [P, nt, 8] buffers (no per-tile copies).
            mxa = outp.tile([P, nt, 8], F32)
            ixa = outp.tile([P, nt, 8], U32)
            for b in range(nb):
                lgT_sb = small.tile([E, TB], F32)
                if b % 2 == 0:
                    nc.scalar.copy(lgT_sb[:], lgT[b][:])
                else:
                    nc.vector.tensor_copy(lgT_sb[:], lgT[b][:])
                for j in range(TB // P):
                    t = b * (TB // P) + j
                    lg_ps = pst.tile([P, E], F32)
                    nc.tensor.matmul(lg_ps[:], lgT_sb[:, j * P:(j + 1) * P],
                                     id_sb[0:E, 0:E], is_transpose=True,
                                     start=True, stop=True)
                    nc.vector.max(mxa[:, t, :], lg_ps[:])
                    nc.vector.max_index(ixa[:, t, :], mxa[:, t, :], lg_ps[:])

            # w1 = sigmoid(l1 - l2), w2 = 1 - w1, batched over all tiles.
            d = outp.tile([P, nt], F32)
            nc.vector.scalar_tensor_tensor(
                d[:], mxa[:, :, 0], 1.0, mxa[:, :, 1], Alu.mult, Alu.subtract)
            owa = outp.tile([P, nt, 2], F32)
            nc.scalar.activation(owa[:, :, 0], d[:],
                                 mybir.ActivationFunctionType.Sigmoid)
            nc.vector.tensor_scalar(owa[:, :, 1], owa[:, :, 0], -1.0, 1.0,
                                    Alu.mult, Alu.add)

            nc.sync.dma_start(ow_ext[:], owa[:])
            nc.scalar.dma_start(oi_ext[:], ixa[:])

    return nc


_NC_CACHE = {}


def _get_nc(tok: int):
    if tok not in _NC_CACHE:
        _NC_CACHE[tok] = build_nc(tok)
    return _NC_CACHE[tok]


def make_in_maps(x: np.ndarray, gate_w: np.ndarray):
    """Shard + split inputs into per-core input maps."""
    xf = np.ascontiguousarray(x.reshape(TOK_TOTAL, H), dtype=np.float32)
    xh = xf.astype(BF)
    xl = (xf - xh.astype(np.float32)).astype(BF)

    w32 = np.asarray(gate_w, np.float32)
    wh = w32.astype(BF)
    wl = (w32 - wh.astype(np.float32)).astype(BF)
    # wt[p, c, s, e] = w_s[e, 128*c + p]
    wt = np.empty((P, NCH, 2, E), dtype=BF)
    wt[:, :, 0, :] = wh.T.reshape(NCH, P, E).transpose(1, 0, 2)
    wt[:, :, 1, :] = wl.T.reshape(NCH, P, E).transpose(1, 0, 2)

    ident = np.eye(P, dtype=np.float32)
    maps = []
    for i in range(N_CORES):
        sl = slice(i * TOK, (i + 1) * TOK)
        # xt[sc, p, cc, s, t] = x_s[token t, 128*(2*sc+cc) + p]
        xt = np.empty((NCH // 2, P, 2, 2, TOK), dtype=BF)
        xhT = np.ascontiguousarray(xh[sl].T).reshape(NCH // 2, 2, P, TOK)
        xlT = np.ascontiguousarray(xl[sl].T).reshape(NCH // 2, 2, P, TOK)
        xt[:, :, :, 0, :] = xhT.transpose(0, 2, 1, 3)
        xt[:, :, :, 1, :] = xlT.transpose(0, 2, 1, 3)
        maps.append({"xt": xt, "wt": wt, "ident": ident})
    return maps


def kernel(x, gate_w, _trace: bool = False):
    x = np.asarray(x, dtype=np.float32)
    gate_w = np.asarray(gate_w, dtype=np.float32)
    nc = _get_nc(TOK)
    in_maps = make_in_maps(x, gate_w)
    res = run_bass_kernel_spmd(
        nc, in_maps, core_ids=list(range(N_CORES)), trace=_trace
    )
    # Device returns [128, nt, 2] partition-major; unpermute to [tok, 2].
    out_w = np.concatenate([
        res.results[i]["out_w"].transpose(1, 0, 2).reshape(TOK, 2)
        for i in range(N_CORES)
    ])
    out_i = np.concatenate([
        res.results[i]["out_i"][:, :, 0:2].transpose(1, 0, 2).reshape(TOK, 2)
        for i in range(N_CORES)
    ])
    topk_weights = out_w.reshape(B, S, 2)
    topk_indices = out_i.astype(np.int32).reshape(B, S, 2)
    if _trace:
        kernel._last_result = res
    return topk_weights, topk_indices


# revision 13
# speedup vs baseline: 1.7759x; 1.0729x over previous
"""MoE router kernel for Trainium2 (8 NeuronCores, SPMD data-parallel).

Computes, for x [B,S,H] and gate_w [E,H]:
    logits = x @ gate_w.T           # [B,S,E]
    p = softmax(logits, -1)
    w, i = top_k(p, 2); w = w / w.sum(-1, keepdims=True)

Math used on-device: renormalized top-2 softmax weights collapse to
    w1 = sigmoid(l1 - l2), w2 = 1 - w1
where l1 >= l2 are the top-2 logits, so the full softmax is never needed.

Sharding: tokens (B*S = 16384) split evenly across 8 cores; gate weights
replicated. Per core: 2048 tokens x 4096 hidden.

v5 design (weight-stationary split-precision bf16, token-group pipeline):
  The host splits x and gate_w into bf16 hi/lo pairs (16 mantissa bits
  total; fp32-grade logits, sigma ~ 4e-6, verified zero top-2 flips on
  the problem distribution) and pre-transposes each core's slice into
  [group, quad, 128, chunk, {hi,lo}, tok] layout with 16 KB contiguous
  per-partition DMA lines.

  Tokens are processed in 4 sequential groups of 512. Per group, the PE
  accumulates logitsT [64, 512] in one PSUM bank over 32 contraction
  chunks (3 bf16 matmuls each: wh*xh + wh*xl + wl*xh, 1 cycle/row).
  Each group's tail (drain, PE back-transpose, DVE max8/max_index,
  sigmoid, output DMA) is emitted inside the NEXT group's stream so it
  hides under DMA/PE; only the last group's tail is exposed.
"""

import sys

sys.path.insert(0, "/opt/trn_rl_repo")

import numpy as np
import ml_dtypes

import concourse.bass as bass
import concourse.mybir as mybir
import concourse.tile as tile
from concourse.bass_utils import run_bass_kernel_spmd
import orjson
import concourse.bass_utils as _bu
import concourse.bass2jax as _b2j

_orig_compile_bir = _bu.compile_bir_kernel


def _legalize_waits(bir_json: bytes) -> bytes:
    """This walrus build allows only ONE sync-wait per compute
    instruction; move excess waits onto a Drain inserted just before
    (Drain accepts many waits)."""
    m = orjson.loads(bir_json)
    changed = False
    for fn in m["functions"]:
        for blk in fn["blocks"]:
            out = []
            for inst in blk["instructions"]:
                si = inst.get("sync_info")
                w = (si or {}).get("on_wait") or []
                if len(w) > 1:
                    for k, wk in enumerate(w[:-1]):
                        out.append({
                            "debug": inst.get("debug", 0),
                            "engine": inst["engine"],
                            "ins": [], "outs": [],
                            "name": inst["name"] + f"-lw{k}",
                            "opcode": "Drain",
                            "sync_info": {"on_update": [], "on_wait": [wk]},
                        })
                    si["on_wait"] = w[-1:]
                    changed = True
                out.append(inst)
            blk["instructions"] = out
    return orjson.dumps(m) if changed else bir_json


def _compile_bir_legalized(bir_json, tmpdir, neff_name="file.neff"):
    return _orig_compile_bir(_legalize_waits(bir_json), tmpdir, neff_name)


_bu.compile_bir_kernel = _compile_bir_legalized
_b2j.compile_bir_kernel = _compile_bir_legalized

F32 = mybir.dt.float32
BF16 = mybir.dt.bfloat16
U32 = mybir.dt.uint32
Alu = mybir.AluOpType
BF = ml_dtypes.bfloat16

B, S, H, E = 4, 4096, 4096, 64
N_CORES = 8
P = 128                      # partitions / tile height
TOK_TOTAL = B * S            # 16384
TOK = TOK_TOTAL // N_CORES   # 2048 tokens per core
NCH = H // P                 # 32 contraction chunks of 128
NG = 4                       # token groups
TG = TOK // NG               # 512 tokens per group (= 1 PSUM bank)
QC = 4                       # DMA quads per group
CCQ = NCH // QC              # 8 chunks per quad
NT = TOK // P                # 16 output tiles of 128 tokens
TPG = TG // P                # 4 tiles per group


def build_nc(tok: int = TOK):
    """Build the per-core Bass program (SPMD: same program, 8 cores)."""
    nc = bass.Bass()

    xt_ext = nc.declare_dram_parameter("xt", [NG, QC, P, CCQ, 2, TG], BF16,
                                       isOutput=False)
    wt_ext = nc.declare_dram_parameter("wt", [P, NCH, 2, E], BF16,
                                       isOutput=False)
    id_ext = nc.declare_dram_parameter("ident", [P, P], F32, isOutput=False)
    ow_ext = nc.declare_dram_parameter("out_w", [P, NT, 2], F32,
                                       isOutput=True)
    oi_ext = nc.declare_dram_parameter("out_i", [P, NT, 8], U32,
                                       isOutput=True)

    with tile.TileContext(nc) as tc:
        with (
            tc.tile_pool(name="consts", bufs=1) as consts,
            tc.tile_pool(name="xin", bufs=4) as xin,
            tc.tile_pool(name="psl", bufs=1, space="PSUM") as psl,
            tc.tile_pool(name="pst", bufs=2, space="PSUM") as pst,
            tc.tile_pool(name="small", bufs=4) as small,
            tc.tile_pool(name="outp", bufs=1) as outp,
        ):
            wt_sb = consts.tile([P, NCH, 2, E], BF16)
            nc.scalar.dma_start(wt_sb[:], wt_ext[:])
            id_sb = consts.tile([P, P], F32)
            nc.scalar.dma_start(id_sb[:], id_ext[:])

            # Primers: walrus allows only ONE sync-wait per compute
            # instruction. Give every engine a first op with no other
            # dependency (const APs are pre-TileContext, untracked), and
            # absorb the wt-DMA sem into a throwaway PE op. The ident-DMA
            # sem rides on the first tail transpose (single wait, arrives
            # long before the tail).
            prim = consts.tile([P, 2], F32)
            nc.vector.memset(prim[:, 0:1], 0.0)
            nc.scalar.copy(prim[:, 1:2], nc.const_aps.tensor(1.0, (P, 1)))
            with tc.tile_pool(name="scr", bufs=1, space="PSUM") as scr_pool:
                scr2 = scr_pool.tile([E, E], BF16)
                nc.tensor.matmul(scr2[:], wt_sb[:, 0, 0, :],
                                 wt_sb[:, 0, 0, :],
                                 is_transpose=True, start=True, stop=True)

            # logitsT accumulators: one [64, TG] bank per token group.
            lgT = [psl.tile([E, TG], F32, name=f"lgT{g}") for g in range(NG)]
            mxa = outp.tile([P, NT, 8], F32)
            ixa = outp.tile([P, NT, 8], U32)
            d_all = outp.tile([P, NT], F32)
            owa = outp.tile([P, NT, 2], F32)

            def emit_quad(g, qc):
                x_sb = xin.tile([P, CCQ, 2, TG], BF16, name="x_sb")
                nc.sync.dma_start(x_sb[:], xt_ext[g, qc])
                for cc in range(CCQ):
                    c = CCQ * qc + cc
                    nc.tensor.matmul(lgT[g][:], wt_sb[:, c, 0, :],
                                     x_sb[:, cc, 0, :],
                                     start=(c == 0), stop=False)
                    nc.tensor.matmul(lgT[g][:], wt_sb[:, c, 0, :],
                                     x_sb[:, cc, 1, :],
                                     start=False, stop=False)
                    nc.tensor.matmul(lgT[g][:], wt_sb[:, c, 1, :],
                                     x_sb[:, cc, 0, :],
                                     start=False, stop=(c == NCH - 1))

            def emit_tail(g):
                lgT_sb = small.tile([E, TG], F32, name="lgT_sb")
                if g % 2 == 0:
                    nc.scalar.copy(lgT_sb[:], lgT[g][:])
                else:
                    nc.vector.tensor_copy(lgT_sb[:], lgT[g][:])
                for j in range(TPG):
                    t = g * TPG + j
                    lg_ps = pst.tile([P, E], F32, name="lg_ps")
                    nc.tensor.matmul(lg_ps[:], lgT_sb[:, j * P:(j + 1) * P],
                                     id_sb[0:E, 0:E], is_transpose=True,
                                     start=True, stop=True)
                    nc.vector.max(mxa[:, t, :], lg_ps[:])
                    nc.vector.max_index(ixa[:, t, :], mxa[:, t, :], lg_ps[:])
                sl = slice(g * TPG, (g + 1) * TPG)
                nc.vector.scalar_tensor_tensor(
                    d_all[:, sl], mxa[:, sl, 0], 1.0, mxa[:, sl, 1],
                    Alu.mult, Alu.subtract)
                nc.scalar.activation(owa[:, sl, 0], d_all[:, sl],
                                     mybir.ActivationFunctionType.Sigmoid)
                nc.vector.tensor_scalar(owa[:, sl, 1], owa[:, sl, 0],
                                        -1.0, 1.0, Alu.mult, Alu.add)
                eng = nc.sync if g % 2 == 0 else nc.scalar
                eng.dma_start(ow_ext[:, sl, :], owa[:, sl, :])
                eng.dma_start(oi_ext[:, sl, :], ixa[:, sl, :])

            for g in range(NG):
                for qc in range(QC):
                    emit_quad(g, qc)
                    # Previous group's tail hides under this stream.
                    if qc == 1 and g >= 1:
                        emit_tail(g - 1)
            emit_tail(NG - 1)

    return nc


_NC_CACHE = {}


def _get_nc(tok: int):
    if tok not in _NC_CACHE:
        _NC_CACHE[tok] = build_nc(tok)
    return _NC_CACHE[tok]


def make_in_maps(x: np.ndarray, gate_w: np.ndarray):
    """Shard + split inputs into per-core input maps."""
    xf = np.ascontiguousarray(x.reshape(TOK_TOTAL, H), dtype=np.float32)
    xh = xf.astype(BF)
    xl = (xf - xh.astype(np.float32)).astype(BF)

    w32 = np.asarray(gate_w, np.float32)
    wh = w32.astype(BF)
    wl = (w32 - wh.astype(np.float32)).astype(BF)
    # wt[p, c, s, e] = w_s[e, 128*c + p]
    wt = np.empty((P, NCH, 2, E), dtype=BF)
    wt[:, :, 0, :] = wh.T.reshape(NCH, P, E).transpose(1, 0, 2)
    wt[:, :, 1, :] = wl.T.reshape(NCH, P, E).transpose(1, 0, 2)

    ident = np.eye(P, dtype=np.float32)
    maps = []
    for i in range(N_CORES):
        sl = slice(i * TOK, (i + 1) * TOK)
        # xt[g, qc, p, cc, s, t] = x_s[token TG*g + t, 128*(CCQ*qc+cc) + p]
        # x_s[sl].T is [H, TOK]; reshape H -> (QC, CCQ, P), TOK -> (NG, TG)
        xhT = np.ascontiguousarray(xh[sl].T).reshape(QC, CCQ, P, NG, TG)
        xlT = np.ascontiguousarray(xl[sl].T).reshape(QC, CCQ, P, NG, TG)
        xt = np.empty((NG, QC, P, CCQ, 2, TG), dtype=BF)
        xt[:, :, :, :, 0, :] = xhT.transpose(3, 0, 2, 1, 4)
        xt[:, :, :, :, 1, :] = xlT.transpose(3, 0, 2, 1, 4)
        maps.append({"xt": xt, "wt": wt, "ident": ident})
    return maps


def kernel(x, gate_w, _trace: bool = False):
    x = np.asarray(x, dtype=np.float32)
    gate_w = np.asarray(gate_w, dtype=np.float32)
    nc = _get_nc(TOK)
    in_maps = make_in_maps(x, gate_w)
    res = run_bass_kernel_spmd(
        nc, in_maps, core_ids=list(range(N_CORES)), trace=_trace
    )
    # Device returns [128, NT, k] partition-major; unpermute to [tok, 2].
    out_w = np.concatenate([
        res.results[i]["out_w"].transpose(1, 0, 2).reshape(TOK, 2)
        for i in range(N_CORES)
    ])
    out_i = np.concatenate([
        res.results[i]["out_i"][:, :, 0:2].transpose(1, 0, 2).reshape(TOK, 2)
        for i in range(N_CORES)
    ])
    topk_weights = out_w.reshape(B, S, 2)
    topk_indices = out_i.astype(np.int32).reshape(B, S, 2)
    if _trace:
        kernel._last_result = res
    return topk_weights, topk_indices
